# revision 1
# baseline (speedup 1.0000x reference)
"""Multi-head attention (B=4, S=2048, D=256, H=4) on 8 trn2 NeuronCores.

Sharding: core c handles batch b = c//2 and query half qh = c%2 (1024
queries), all 4 heads, full 2048 keys.  Inputs are pre-transposed on the
host (x[b].T and W.T) so every matmul contraction runs with the
contracted dim on SBUF partitions and no on-device transposes are
needed.

Per-core dataflow (scores kept transposed: [keys, queries]):
  QT = WQT.T-chunks @ xq          -> Q.T   [256(feat), 1024(q)]
  KT = WKT.T-chunks @ xT          -> K.T   [256(feat), 2048(k)]
  V  = xT-chunks.T  @ WVT         -> V_aug [2048(k), 4, 65]  (ones col)
  per head pair p, query half f, key tile kt (128 keys):
     S.T[kt, q] = KT_h-slices.T @ QT-slices  (2 heads row-packed in PE)
     E.T        = exp(S.T * scale + mask_bias[key])      (ScalarE)
     cd_h      += V_aug_h.T @ E.T   (rows 0-63 = ctx.T, row 64 = den;
                                     one PSUM bank per head, accumulated
                                     over the 16 key tiles)
  ctx normalized by 1/den (PE broadcast of reciprocal + DVE multiply)
  out = sum_h ctxn_h.T-chunks @ WOT_h  -> [1024(q), 256]

Matmul operands are float32r (TF32-like, 1 PE cycle/col for N>=256 vs 4
cycles for fp32's LOW_HIGH two-pass mode).  fp32r matmuls must write
PSUM at partition offset 0 (ISA rule s3d3_mm_valid_dst_partition), which
is why each head's ctx accumulator lives in its own bank instead of
being column-packed.  The reciprocal/broadcast path stays full fp32.
"""

import sys

for _p in ("/opt/trn_rl_repo",):
    if _p not in sys.path:
        sys.path.insert(0, _p)

import numpy as np

B, S, D, H, HD = 4, 2048, 256, 4, 64
SCALE = HD**-0.5
NCORES = 8
QS = S // 2  # queries per core
QH = QS // 2  # query half (one psum bank wide per head)
P = 128
NKT = S // P  # 16 key tiles

_cache = {}


def _build_nc():
    import concourse.mybir as mybir
    from concourse import bacc
    from concourse.tile import TileContext

    f32 = mybir.dt.float32
    f32r = mybir.dt.float32r
    Exp = mybir.ActivationFunctionType.Exp
    Ln = getattr(mybir.ActivationFunctionType, 'Ln', None) or mybir.ActivationFunctionType.Log

    nc = bacc.Bacc("TRN2", target_bir_lowering=False, debug=False)

    xT_d = nc.dram_tensor("xT", [D, S], f32, kind="ExternalInput")
    xq_d = nc.dram_tensor("xq", [D, QS], f32, kind="ExternalInput")
    wqt_d = nc.dram_tensor("wqt", [D, D], f32, kind="ExternalInput")
    wkt_d = nc.dram_tensor("wkt", [D, D], f32, kind="ExternalInput")
    wvt_d = nc.dram_tensor("wvt", [D, D], f32, kind="ExternalInput")
    wot_d = nc.dram_tensor("wot", [D, D], f32, kind="ExternalInput")
    bias_d = nc.dram_tensor("bias", [P, NKT], f32, kind="ExternalInput")
    out_d = nc.dram_tensor("out", [QS, D], f32, kind="ExternalOutput")

    with TileContext(nc) as tc:
        with (
            tc.tile_pool(name="const", bufs=1) as const,
            tc.tile_pool(name="big", bufs=1) as big,
            tc.tile_pool(name="et", bufs=6) as etp,
            tc.tile_pool(name="small", bufs=2) as small,
            tc.tile_pool(name="psA", bufs=2, space="PSUM") as psA,
            tc.tile_pool(name="psCD", bufs=2, space="PSUM") as psCD,
        ):
            # ---- constants / weights / bias ----
            ones4 = const.tile([P, 4], f32)
            nc.vector.memset(ones4, 1.0)
            ones_row_f = const.tile([65, P], f32)
            nc.vector.memset(ones_row_f, 1.0)
            ones_row = const.tile([65, P], f32r)
            nc.vector.tensor_copy(ones_row, ones_row_f)
            bias_sb = const.tile([P, NKT], f32)
            nc.gpsimd.dma_start(out=bias_sb, in_=bias_d[:, :])

            # spread input DMAs over four engine queues so the issues overlap;
            # Q-path inputs (wqt, xq) first so projections start early.
            w_sb = {}
            w_engines = {"wqt": nc.scalar, "wkt": nc.gpsimd, "wvt": nc.gpsimd}
            # (wqt then xq then xT all on the scalar HWDGE queue: the DMA
            # engines round-robin across queues, so sharing one queue is
            # the only way to prioritize the Q-path inputs)
            for nm, dram in (("wqt", wqt_d), ("wkt", wkt_d), ("wvt", wvt_d)):
                wt = const.tile([P, 2, D], f32r, name=f"w_{nm}", tag=f"w_{nm}")
                w_engines[nm].dma_start(
                    out=wt, in_=dram.rearrange("(c p) e -> p c e", p=P).bitcast(f32r)
                )
                w_sb[nm] = wt
            xq_sb = []
            for c in range(2):
                xq = big.tile([P, QS], f32r, name=f"xq{c}", tag=f"xq{c}")
                [nc.scalar, nc.scalar][c].dma_start(
                    out=xq, in_=xq_d[c * P : (c + 1) * P, :].bitcast(f32r)
                )
                xq_sb.append(xq)
            xT_sb = []
            for c in range(2):
                xt = big.tile([P, S], f32r, name=f"xT{c}", tag=f"xT{c}")
                nc.scalar.dma_start(out=xt, in_=xT_d[c * P : (c + 1) * P, :].bitcast(f32r))
                xT_sb.append(xt)
            # W_O.T grouped per head: [64, 4, 256] so each head's contraction
            # chunk starts at partition 0.
            wot_sb = const.tile([64, 4, D], f32r, name="w_wot", tag="w_wot")
            nc.gpsimd.dma_start(
                out=wot_sb, in_=wot_d.rearrange("(h p) e -> p h e", p=64).bitcast(f32r)
            )

            # ---- projections (emitted lazily so they interleave with
            # attention: the Tile scheduler + in-order engines execute
            # roughly in emission order, and the ScalarE-bound attention
            # steady state leaves PE gaps that this work fills, which also
            # keeps the PE's HAM clock-gate warm) ----
            QT_sb = [None, None]
            KT_sb = [None, None]
            V_sb = [None] * NKT
            ctxn_sb = []
            for h in range(H):
                cn = big.tile([64, QS], f32r, name=f"ctxn{h}", tag=f"ctxn{h}")
                ctxn_sb.append(cn)

            def qt_proj(m):
                qt = big.tile([P, QS], f32r, name=f"QT{m}", tag=f"QT{m}")
                ps = psA.tile([P, 1024], f32, name="psA", tag="psA")
                for n in range(QS // 512):
                    for c in range(2):
                        nc.tensor.matmul(
                            ps[:, n * 512 : (n + 1) * 512],
                            w_sb["wqt"][:, c, m * P : (m + 1) * P],
                            xq_sb[c][:, n * 512 : (n + 1) * 512],
                            start=(c == 0),
                            stop=(c == 1),
                        )
                nc.vector.tensor_copy(qt, ps)
                QT_sb[m] = qt

            def kt_proj(m, half):
                if KT_sb[m] is None:
                    KT_sb[m] = big.tile([P, S], f32r, name=f"KT{m}", tag=f"KT{m}")
                kt_t = KT_sb[m]
                ps = psA.tile([P, 1024], f32, name="psA", tag="psA")
                for n in range(2):
                    for c in range(2):
                        nc.tensor.matmul(
                            ps[:, n * 512 : (n + 1) * 512],
                            w_sb["wkt"][:, c, m * P : (m + 1) * P],
                            xT_sb[c][:, half * 1024 + n * 512 : half * 1024 + (n + 1) * 512],
                            start=(c == 0),
                            stop=(c == 1),
                        )
                nc.vector.tensor_copy(kt_t[:, half * 1024 : (half + 1) * 1024], ps)

            def v_proj(mt):
                # V_aug [s-tile, 4, 65]: per-head 64 value cols + a ones col
                # (whose cd-matmul row is the softmax denominator).
                vt = big.tile([P, 4, 65], f32r, name=f"V{mt}", tag=f"V{mt}")
                ps = psCD.tile([P, 512], f32, name="ps_v", tag="psCD")
                for c in range(2):
                    nc.tensor.matmul(
                        ps[:, :D],
                        xT_sb[c][:, mt * P : (mt + 1) * P],
                        w_sb["wvt"][:, c, :],
                        start=(c == 0),
                        stop=(c == 1),
                    )
                nc.vector.tensor_copy(
                    vt[:, :, 0:64], ps[:, :D].rearrange("p (h e) -> p h e", h=4)
                )
                nc.vector.tensor_copy(vt[:, :, 64], ones4)
                V_sb[mt] = vt

            def kt_loop(p, f, inject=None):
                # rows 0-63: ctx.T for head 2p+h2; row 64: denominator.
                # One bank per head (h2 chooses the 512-col half).
                ps_cd = psCD.tile([65, 1024], f32, name="ps_cd", tag="psCD")
                for kt in range(NKT):
                    ps_s = psA.tile([P, 1024], f32, name="ps_s", tag="psA")
                    # warming matmul: same operands as the h2=0 scores matmul,
                    # overwritten by it (start=True clears the bank).  Fills
                    # the PE's ScalarE-wait gaps so the HAM activity monitor
                    # keeps the PE clock at 2.4GHz instead of oscillating
                    # down to 1.2GHz (which would make PE the bottleneck).
                    nc.tensor.matmul(
                        ps_s[:, 0:QH],
                        KT_sb[p][0:64, kt * P : (kt + 1) * P],
                        QT_sb[p][0:64, f * QH : (f + 1) * QH],
                        start=True,
                        stop=True,
                        tile_position=(0, 0),
                    )
                    for h2 in range(2):
                        nc.tensor.matmul(
                            ps_s[:, h2 * 512 : h2 * 512 + QH],
                            KT_sb[p][64 * h2 : 64 * h2 + 64, kt * P : (kt + 1) * P],
                            QT_sb[p][64 * h2 : 64 * h2 + 64, f * QH : (f + 1) * QH],
                            start=True,
                            stop=True,
                            tile_position=(64 * h2, 0),
                        )
                    et = etp.tile([P, 1024], f32r, name="et", tag="et")
                    nc.scalar.activation(
                        et, ps_s, Exp, bias=bias_sb[:, kt : kt + 1], scale=SCALE
                    )
                    for h2 in range(2):
                        h = 2 * p + h2
                        nc.tensor.matmul(
                            ps_cd[0:65, h2 * 512 : h2 * 512 + QH],
                            V_sb[kt][:, h, :],
                            et[:, h2 * 512 : h2 * 512 + QH],
                            start=(kt == 0),
                            stop=(kt == NKT - 1),
                        )
                    if inject and kt in inject:
                        inject[kt]()
                return ps_cd

            def finish_cd(ps_cd):
                # Emitted right after a section's kt-loop: evict ctx+den to
                # SBUF and take the reciprocal of the den row (DVE, 6.5us for
                # a single-partition row, overlapped with the next section).
                # Releases the PSUM slot one section early.
                cdsb = small.tile([65, 1024], f32, name="cdsb", tag="cdsb")
                nc.vector.tensor_copy(cdsb, ps_cd)
                recip = small.tile([65, 1024], f32r, name="recip", tag="recip")
                with nc.allow_low_precision(reason="f32r rounding of 1/den"):
                    nc.vector.reciprocal(recip[64:65, :], ps_cd[64:65, :])
                return cdsb, recip

            def apply_norm(p, f, fin):
                # Emitted two sections later (so the reciprocal is long done
                # and the PE broadcast can't head-of-line-block anything):
                # PE row-broadcast of 1/den, then DVE multiplies write the
                # normalized ctx.T to its per-head SBUF tile.
                cdsb, recip = fin
                ps_r = psA.tile([P, 1024], f32, name="ps_r", tag="psA")
                for h2 in range(2):
                    nc.tensor.matmul(
                        ps_r[:, h2 * 512 : h2 * 512 + QH],
                        ones_row[64:65, :],
                        recip[64:65, h2 * 512 : h2 * 512 + QH],
                        start=True,
                        stop=True,
                        tile_position=(64, 0),
                    )
                r_sb = small.tile([P, 1024], f32, name="r_sb", tag="r_sb")
                nc.vector.tensor_copy(r_sb, ps_r)
                for h2 in range(2):
                    nc.vector.tensor_mul(
                        ctxn_sb[2 * p + h2][:, f * QH : (f + 1) * QH],
                        cdsb[0:64, h2 * 512 : h2 * 512 + QH],
                        r_sb[0:64, h2 * 512 : h2 * 512 + QH],
                    )

            def oproj(m):
                # contract over 4 per-head chunks of 64
                ps = psCD.tile([P, 512], f32, name="ps_o", tag="psCD")
                for h in range(H):
                    nc.tensor.matmul(
                        ps[:, :D],
                        ctxn_sb[h][:, m * P : (m + 1) * P],
                        wot_sb[:, h, :],
                        start=(h == 0),
                        stop=(h == H - 1),
                    )
                ot = small.tile([P, D], f32, name="ot", tag="ot")
                nc.vector.tensor_copy(ot, ps[:, :D])
                nc.sync.dma_start(out=out_d[m * P : (m + 1) * P, :], in_=ot)

            # prologue: only what the first section needs immediately
            qt_proj(0)
            kt_proj(0, 0)
            kt_proj(0, 1)
            for mt in range(3):
                v_proj(mt)

            # section (0,0): stream remaining V tiles 2 steps ahead of their
            # cd-use; pair-1 Q/K projections fill later steps.
            inj00 = {kt: (lambda mt=kt + 2: v_proj(mt)) for kt in range(1, NKT - 2)}
            inj00[NKT - 2] = lambda: qt_proj(1)
            inj00[NKT - 1] = lambda: kt_proj(1, 0)
            cd00 = kt_loop(0, 0, inj00)
            fin00 = finish_cd(cd00)
            cd10 = kt_loop(1, 0, {1: lambda: kt_proj(1, 1)})
            fin10 = finish_cd(cd10)
            cd01 = kt_loop(0, 1, {0: lambda: apply_norm(0, 0, fin00)})
            fin01 = finish_cd(cd01)
            cd11 = kt_loop(
                1,
                1,
                {
                    0: lambda: apply_norm(1, 0, fin10),
                    4: lambda: oproj(0),
                    6: lambda: oproj(1),
                    8: lambda: oproj(2),
                    10: lambda: oproj(3),
                },
            )
            fin11 = finish_cd(cd11)
            apply_norm(0, 1, fin01)
            apply_norm(1, 1, fin11)
            for m in range(4, 8):
                oproj(m)

    nc.compile()
    return nc


def _get_nc():
    if "nc" not in _cache:
        _cache["nc"] = _build_nc()
    return _cache["nc"]


def make_in_maps(x, W_Q, W_K, W_V, W_O, mask):
    wqt = np.ascontiguousarray(W_Q.T).astype(np.float32)
    wkt = np.ascontiguousarray(W_K.T).astype(np.float32)
    wvt = np.ascontiguousarray(W_V.T).astype(np.float32)
    wot = np.ascontiguousarray(W_O.T).astype(np.float32)
    in_maps = []
    for c in range(NCORES):
        b, qh = c // 2, c % 2
        xT_b = np.ascontiguousarray(np.asarray(x[b]).T).astype(np.float32)
        xq = np.ascontiguousarray(xT_b[:, qh * QS : (qh + 1) * QS])
        bias = np.where(np.asarray(mask[b]) == 0, -1e30, 0.0).astype(np.float32)
        bias = np.ascontiguousarray(bias.reshape(NKT, P).T)
        in_maps.append(
            {
                "xT": xT_b,
                "xq": xq,
                "wqt": wqt,
                "wkt": wkt,
                "wvt": wvt,
                "wot": wot,
                "bias": bias,
            }
        )
    return in_maps


def gather(results):
    out = np.empty((B, S, D), np.float32)
    for c in range(NCORES):
        b, qh = c // 2, c % 2
        out[b, qh * QS : (qh + 1) * QS, :] = results[c]["out"]
    return out


def kernel(x, W_Q, W_K, W_V, W_O, mask):
    from concourse.bass_utils import run_bass_kernel_spmd

    nc = _get_nc()
    in_maps = make_in_maps(x, W_Q, W_K, W_V, W_O, mask)
    res = run_bass_kernel_spmd(nc, in_maps, core_ids=list(range(NCORES)))
    return gather(res.results)



# revision 10
# speedup vs baseline: 1.4785x; 1.4785x over previous
"""Multi-head attention (B=4, S=2048, D=256, H=4) on 8 trn2 NeuronCores.

Sharding: core c handles batch b = c//2 and query half qh = c%2 (1024
queries), all 4 heads, full 2048 keys.  Inputs are pre-transposed on the
host (x[b].T and W.T) and converted to bf16 (halves the HBM prologue
traffic; the 8-bit mantissa is noise at the 2e-2 gate), so every matmul
contraction runs with the contracted dim on SBUF partitions and no
on-device transposes are needed.  The host additionally rotates the key
axis per core so the core's own query half occupies columns 0:1024 of
xT (softmax is permutation-invariant over keys; the mask bias is
rotated identically).  That removes a separate xq input: the Q
projection reads xT[:, 0:1024] and starts as soon as the first half of
xT lands.

Per-core dataflow (scores kept transposed: [keys, queries]):
  QT = WQT.T-chunks @ xT[:, :1024]  -> Q.T   [256(feat), 1024(q)]
  KT = WKT.T-chunks @ xT            -> K.T   [256(feat), 2048(k)]
  V  = xT-chunks.T  @ WVT           -> V_aug [2048(k), 4, 65] (ones col)
  per head pair p, query half f, key tile kt (128 keys):
     S.T[kt, q] = KT_h-slices.T @ QT-slices  (2 heads row-packed in PE,
                                              concurrent via row groups)
     E.T        = exp(S.T * scale + mask_bias[key])      (ScalarE)
     cd_h      += V_aug_h.T @ E.T   (rows 0-63 = ctx.T, row 64 = den;
                                     one PSUM bank per head, accumulated
                                     over the 16 key tiles)
  1/den via the DVE bit-trick reciprocal_approx_fast on the single den
  row (~1.3us instead of 6.5us for the exact iterative-divide
  reciprocal, whose DVE head-of-line blocking stalled the PE >3.4us at
  a section boundary and knocked the HAM clock gate to K=4/8 for the
  rest of the kernel), then a DVE copy rounds it to f32r (a bitcast
  view fails BIR verification; a direct f32r write from the custom DVE
  op produces garbage).  ctx normalized by 1/den (PE row-broadcast +
  DVE mul, written as bf16 so the O projection runs bf16).
  out = sum_h ctxn_h.T-chunks @ WOT_h  -> [1024(q), 256]

Attention-core matmul operands are float32r (TF32-like, 1 PE cycle/col
for N>=256); projections are bf16 end to end.  fp32r matmuls must write
PSUM at partition offset 0, which is why each head's ctx accumulator
lives in its own bank instead of being column-packed.  No PE "warming"
filler matmuls: profiling showed the HAM throttle flips on duty cycle,
not just long gaps, and sticks cold for tens of us; filler only adds
cycles to the cold-clock critical path.
"""

import sys

for _p in ("/opt/trn_rl_repo",):
    if _p not in sys.path:
        sys.path.insert(0, _p)

import ml_dtypes
import numpy as np

B, S, D, H, HD = 4, 2048, 256, 4, 64
SCALE = HD**-0.5
NCORES = 8
QS = S // 2  # queries per core
QH = QS // 2  # query half (one psum bank wide per head)
P = 128
NKT = S // P  # 16 key tiles

_cache = {}


def _build_nc():
    import concourse.mybir as mybir
    from concourse import bacc
    from concourse.dve_ops import RECIP_APPROX_FAST_CONSTS, RECIPROCAL_APPROX_FAST
    from concourse.tile import TileContext

    f32 = mybir.dt.float32
    f32r = mybir.dt.float32r
    bf16 = mybir.dt.bfloat16
    Exp = mybir.ActivationFunctionType.Exp

    nc = bacc.Bacc("TRN2", target_bir_lowering=False, debug=False)

    xT_d = nc.dram_tensor("xT", [D, S], bf16, kind="ExternalInput")
    wqt_d = nc.dram_tensor("wqt", [D, D], bf16, kind="ExternalInput")
    wkt_d = nc.dram_tensor("wkt", [D, D], bf16, kind="ExternalInput")
    wvt_d = nc.dram_tensor("wvt", [D, D], bf16, kind="ExternalInput")
    wot_d = nc.dram_tensor("wot", [D, D], bf16, kind="ExternalInput")
    bias_d = nc.dram_tensor("bias", [P, NKT], f32, kind="ExternalInput")
    out_d = nc.dram_tensor("out", [QS, D], f32, kind="ExternalOutput")

    with TileContext(nc) as tc:
        with (
            tc.tile_pool(name="const", bufs=1) as const,
            tc.tile_pool(name="big", bufs=1) as big,
            tc.tile_pool(name="et", bufs=6) as etp,
            tc.tile_pool(name="small", bufs=2) as small,
            tc.tile_pool(name="psA", bufs=2, space="PSUM") as psA,
            tc.tile_pool(name="psCD", bufs=2, space="PSUM") as psCD,
        ):
            # ---- constants / weights / bias ----
            ones4 = const.tile([P, 4], f32)
            nc.vector.memset(ones4, 1.0)
            ones_row_f = const.tile([65, P], f32)
            nc.vector.memset(ones_row_f, 1.0)
            ones_row = const.tile([65, P], f32r)
            nc.vector.tensor_copy(ones_row, ones_row_f)

            # Input DMAs.  SP HWDGE ring: mask bias (tiny, needed by the
            # first activation) then the xT column-halves, query half first
            # so the Q/K projections start early.  ACT HWDGE ring: weights
            # in first-use order.  The two rings round-robin at packet
            # granularity across the 16 SDMA engines.
            bias_sb = const.tile([P, NKT], f32)
            nc.sync.dma_start(out=bias_sb, in_=bias_d[:, :])
            xT_sb = []
            for c in range(2):
                xt = big.tile([P, S], bf16, name=f"xT{c}", tag=f"xT{c}")
                xT_sb.append(xt)
            for half in range(2):
                for c in range(2):
                    nc.sync.dma_start(
                        out=xT_sb[c][:, half * QS : (half + 1) * QS],
                        in_=xT_d[c * P : (c + 1) * P, half * QS : (half + 1) * QS],
                    )
            w_sb = {}
            for nm, dram in (("wqt", wqt_d), ("wkt", wkt_d), ("wvt", wvt_d)):
                wt = const.tile([P, 2, D], bf16, name=f"w_{nm}", tag=f"w_{nm}")
                nc.scalar.dma_start(
                    out=wt, in_=dram.rearrange("(c p) e -> p c e", p=P)
                )
                w_sb[nm] = wt
            # W_O.T grouped per head: [64, 4, 256] so each head's contraction
            # chunk starts at partition 0.
            wot_sb = const.tile([64, 4, D], bf16, name="w_wot", tag="w_wot")
            nc.scalar.dma_start(
                out=wot_sb, in_=wot_d.rearrange("(h p) e -> p h e", p=64)
            )

            # ---- projections (emitted lazily so they interleave with
            # attention: the Tile scheduler + in-order engines execute
            # roughly in emission order, and the ScalarE-bound attention
            # steady state leaves PE gaps that this work fills) ----
            QT_sb = [None, None]
            KT_sb = [None, None]
            V_sb = [None] * NKT
            ctxn_sb = []
            for h in range(H):
                cn = big.tile([64, QS], bf16, name=f"ctxn{h}", tag=f"ctxn{h}")
                ctxn_sb.append(cn)

            def qt_proj(m):
                qt = big.tile([P, QS], f32r, name=f"QT{m}", tag=f"QT{m}")
                ps = psA.tile([P, 1024], f32, name="psA", tag="psA")
                for n in range(QS // 512):
                    for c in range(2):
                        nc.tensor.matmul(
                            ps[:, n * 512 : (n + 1) * 512],
                            w_sb["wqt"][:, c, m * P : (m + 1) * P],
                            xT_sb[c][:, n * 512 : (n + 1) * 512],
                            start=(c == 0),
                            stop=(c == 1),
                        )
                nc.vector.tensor_copy(qt, ps)
                QT_sb[m] = qt

            def kt_proj(m, half):
                if KT_sb[m] is None:
                    KT_sb[m] = big.tile([P, S], f32r, name=f"KT{m}", tag=f"KT{m}")
                kt_t = KT_sb[m]
                ps = psA.tile([P, 1024], f32, name="psA", tag="psA")
                for n in range(2):
                    for c in range(2):
                        nc.tensor.matmul(
                            ps[:, n * 512 : (n + 1) * 512],
                            w_sb["wkt"][:, c, m * P : (m + 1) * P],
                            xT_sb[c][:, half * 1024 + n * 512 : half * 1024 + (n + 1) * 512],
                            start=(c == 0),
                            stop=(c == 1),
                        )
                nc.vector.tensor_copy(kt_t[:, half * 1024 : (half + 1) * 1024], ps)

            def v_proj(mt):
                # V_aug [s-tile, 4, 65]: per-head 64 value cols + a ones col
                # (whose cd-matmul row is the softmax denominator).
                vt = big.tile([P, 4, 65], f32r, name=f"V{mt}", tag=f"V{mt}")
                ps = psA.tile([P, 512], f32, name="ps_v", tag="psA")
                for c in range(2):
                    nc.tensor.matmul(
                        ps[:, :D],
                        xT_sb[c][:, mt * P : (mt + 1) * P],
                        w_sb["wvt"][:, c, :],
                        start=(c == 0),
                        stop=(c == 1),
                    )
                nc.vector.tensor_copy(
                    vt[:, :, 0:64], ps[:, :D].rearrange("p (h e) -> p h e", h=4)
                )
                nc.vector.tensor_copy(vt[:, :, 64], ones4)
                V_sb[mt] = vt

            def finish_cd(ps_cd):
                # Emitted right after a section's last cd matmul.  The
                # approximate reciprocal goes FIRST on the DVE queue (~18
                # correct bits, far beyond the 2e-2 gate) so the next
                # section's norm broadcast can't stall the PE.  It runs over
                # all 65 rows because the custom DVE op mis-addresses when
                # its APs start at a non-zero base partition (hardware-
                # verified): rows 0-63 compute throwaway reciprocals of ctx,
                # row 64 is the denominator reciprocal we use.  The copy
                # rounds row 64 to f32r for the broadcast matmul (a bitcast
                # view fails BIR verification); the ctx+den eviction then
                # frees the PSUM slot.
                recip_f = small.tile([65, 1024], f32, name="recip_f", tag="recip_f")
                ck = RECIP_APPROX_FAST_CONSTS
                nc.vector._custom_dve(
                    RECIPROCAL_APPROX_FAST,
                    out=recip_f[0:65, :],
                    in0=ps_cd[0:65, :],
                    s0=ck["s0"],
                    s1=ck["s1"],
                    imm2=ck["imm2"],
                )
                recip = small.tile([65, 1024], f32r, name="recip", tag="recip")
                nc.vector.tensor_copy(recip[64:65, :], recip_f[64:65, :])
                cdsb = small.tile([65, 1024], f32, name="cdsb", tag="cdsb")
                nc.vector.tensor_copy(cdsb, ps_cd)
                return cdsb, recip

            def norm_bc(recip):
                # PE row-broadcast of the 1/den row to all 128 partitions.
                ps_r = psA.tile([P, 1024], f32, name="ps_r", tag="psA")
                for h2 in range(2):
                    nc.tensor.matmul(
                        ps_r[:, h2 * 512 : h2 * 512 + QH],
                        ones_row[64:65, :],
                        recip[64:65, h2 * 512 : h2 * 512 + QH],
                        start=True,
                        stop=True,
                        tile_position=(64, 0),
                    )
                r_sb = small.tile([P, 1024], f32, name="r_sb", tag="r_sb")
                nc.vector.tensor_copy(r_sb, ps_r)
                return r_sb

            def apply_norm(p, f, fin):
                # Emitted early in the NEXT section (the fast reciprocal is
                # done ~2.5us after the section boundary): broadcast, then
                # DVE multiplies write the normalized ctx.T (bf16) to its
                # per-head SBUF tile.
                cdsb, recip = fin
                r_sb = norm_bc(recip)
                for h2 in range(2):
                    nc.vector.tensor_mul(
                        ctxn_sb[2 * p + h2][:, f * QH : (f + 1) * QH],
                        cdsb[0:64, h2 * 512 : h2 * 512 + QH],
                        r_sb[0:64, h2 * 512 : h2 * 512 + QH],
                    )

            def oproj(m):
                # contract over 4 per-head chunks of 64
                ps = psA.tile([P, 512], f32, name="ps_o", tag="psA")
                for h in range(H):
                    nc.tensor.matmul(
                        ps[:, :D],
                        ctxn_sb[h][:, m * P : (m + 1) * P],
                        wot_sb[:, h, :],
                        start=(h == 0),
                        stop=(h == H - 1),
                    )
                ot = small.tile([P, D], f32, name="ot", tag="ot")
                nc.vector.tensor_copy(ot, ps[:, :D])
                nc.sync.dma_start(out=out_d[m * P : (m + 1) * P, :], in_=ot)

            # ---- flat software-pipelined schedule over all 4 sections ----
            # Emission order per step i: scores+exp for step i+1, THEN the
            # cd matmuls for step i.  On the in-order PE queue this puts
            # scores(i+1) AHEAD of cd(i) (which must wait for exp(i)), so
            # the act-to-act critical cycle is just scores+sem instead of
            # act->cd->scores (~200ns/step saved and section boundaries
            # pipeline for free).
            SECS = [(0, 0), (1, 0), (0, 1), (1, 1)]
            FL = [(si, kt) for si in range(4) for kt in range(NKT)]
            ps_cds = [None] * 4
            fins = [None] * 4
            ets = {}

            def scores_act(i):
                si, kt = FL[i]
                p, f = SECS[si]
                ps_s = psA.tile([P, 1024], f32, name="ps_s", tag="psA")
                for h2 in range(2):
                    nc.tensor.matmul(
                        ps_s[:, h2 * 512 : h2 * 512 + QH],
                        KT_sb[p][64 * h2 : 64 * h2 + 64, kt * P : (kt + 1) * P],
                        QT_sb[p][64 * h2 : 64 * h2 + 64, f * QH : (f + 1) * QH],
                        start=True,
                        stop=True,
                        tile_position=(64 * h2, 0),
                    )
                et = etp.tile([P, 1024], f32r, name="et", tag="et")
                nc.scalar.activation(
                    et, ps_s, Exp, bias=bias_sb[:, kt : kt + 1], scale=SCALE
                )
                ets[i] = et

            def cd_step(i):
                si, kt = FL[i]
                p, f = SECS[si]
                if kt == 0:
                    ps_cds[si] = psCD.tile([65, 1024], f32, name="ps_cd", tag="psCD")
                et = ets.pop(i)
                for h2 in range(2):
                    nc.tensor.matmul(
                        ps_cds[si][0:65, h2 * 512 : h2 * 512 + QH],
                        V_sb[kt][:, 2 * p + h2, :],
                        et[:, h2 * 512 : h2 * 512 + QH],
                        start=(kt == 0),
                        stop=(kt == NKT - 1),
                    )
                if kt == NKT - 1:
                    fins[si] = finish_cd(ps_cds[si])

            inj = {
                (0, 2): [lambda: kt_proj(0, 1)],
                (0, 11): [lambda: qt_proj(1)],
                (0, 12): [lambda: kt_proj(1, 0)],
                (1, 1): [lambda: kt_proj(1, 1)],
                (1, 3): [lambda: apply_norm(0, 0, fins[0])],
                (2, 1): [lambda: apply_norm(1, 0, fins[1])],
                (2, 5): [lambda: oproj(0)],
                (2, 10): [lambda: oproj(1)],
                (3, 1): [lambda: apply_norm(0, 1, fins[2])],
                (3, 5): [lambda: oproj(2)],
                (3, 10): [lambda: oproj(3)],
            }
            # stream V tiles two steps ahead of their cd-use
            for k in range(0, NKT - 2):
                inj.setdefault((0, k), []).insert(0, lambda mt=k + 2: v_proj(mt))

            # prologue: only what the first steps need immediately
            qt_proj(0)
            kt_proj(0, 0)
            v_proj(0)
            v_proj(1)

            scores_act(0)
            for i in range(len(FL)):
                if i + 1 < len(FL):
                    scores_act(i + 1)
                cd_step(i)
                for fn in inj.get(FL[i], []):
                    fn()
            fin11 = fins[3]

            # epilogue: chunked so each output DMA starts as soon as its
            # quarter of the normalized ctx is ready.
            cdsb11, recip11 = fin11
            r_sb11 = norm_bc(recip11)
            for mq in range(2):
                cols = slice(512 + mq * 256, 512 + (mq + 1) * 256)
                for h2 in range(2):
                    nc.vector.tensor_mul(
                        ctxn_sb[2 + h2][:, cols],
                        cdsb11[0:64, h2 * 512 + mq * 256 : h2 * 512 + (mq + 1) * 256],
                        r_sb11[0:64, h2 * 512 + mq * 256 : h2 * 512 + (mq + 1) * 256],
                    )
                oproj(4 + 2 * mq)
                oproj(5 + 2 * mq)

    nc.compile()
    return nc


def _get_nc():
    if "nc" not in _cache:
        _cache["nc"] = _build_nc()
    return _cache["nc"]


def make_in_maps(x, W_Q, W_K, W_V, W_O, mask):
    bf = ml_dtypes.bfloat16
    wqt = np.ascontiguousarray(W_Q.T).astype(bf)
    wkt = np.ascontiguousarray(W_K.T).astype(bf)
    wvt = np.ascontiguousarray(W_V.T).astype(bf)
    wot = np.ascontiguousarray(W_O.T).astype(bf)
    in_maps = []
    for c in range(NCORES):
        b, qh = c // 2, c % 2
        xT_b = np.asarray(x[b]).T.astype(np.float32)
        bias_row = np.where(np.asarray(mask[b]) == 0, -1e30, 0.0).astype(np.float32)
        if qh:
            # rotate keys so this core's query half sits in columns 0:QS
            xT_b = np.concatenate([xT_b[:, QS:], xT_b[:, :QS]], axis=1)
            bias_row = np.concatenate([bias_row[QS:], bias_row[:QS]])
        bias = np.ascontiguousarray(bias_row.reshape(NKT, P).T)
        in_maps.append(
            {
                "xT": np.ascontiguousarray(xT_b).astype(bf),
                "wqt": wqt,
                "wkt": wkt,
                "wvt": wvt,
                "wot": wot,
                "bias": bias,
            }
        )
    return in_maps


def gather(results):
    out = np.empty((B, S, D), np.float32)
    for c in range(NCORES):
        b, qh = c // 2, c % 2
        out[b, qh * QS : (qh + 1) * QS, :] = results[c]["out"]
    return out


def kernel(x, W_Q, W_K, W_V, W_O, mask):
    from concourse.bass_utils import run_bass_kernel_spmd

    nc = _get_nc()
    in_maps = make_in_maps(x, W_Q, W_K, W_V, W_O, mask)
    res = run_bass_kernel_spmd(nc, in_maps, core_ids=list(range(NCORES)))
    return gather(res.results)
